# revision 7
# baseline (speedup 1.0000x reference)
"""Distributed attention kernel for 8 TRN2 NeuronCores.

Problem: L=2048, B=2, E=256, H=8 heads, D=32 head-dim, fp32.
Sharding: DP2 over batch x TP4 over heads (2 heads/core). Row-parallel
output projection with a chunked ReduceScatter over each 4-core group.

Per-core pipeline:
  phase 0: DMA in pre-transposed bf16 x.T shards + bf16 weight slices.
  phase 1: q.T/k.T = W^T @ x.T (bf16 matmul, f32 psum, bias fused in the
           psum->sbuf copy); v = value @ Wv (natural layout) interleaved
           with a ones-column for the softmax row-sum.
  phase 2: per tq-chunk(512) / head / tk-pair: S.T = k.T^T q.T in
           float32r (full-rate fp32), exp on ScalarE with fused 1/sqrt(D)
           scale (psum->sbuf, bf16), P.T @ [v|1] accumulates O.T plus the
           row-sums Z in one psum tile; normalize by 1/Z fused into the
           psum->sbuf copy; out-proj (row-parallel) + bias; DMA to a DRAM
           bounce buffer.
  phase 3: ReduceScatter(add) over the TP group per tq-chunk, DMA to out.
"""

import os
import sys

import numpy as np

for _p in ("/opt/trn_rl_repo",):
    if _p not in sys.path and os.path.isdir(_p):
        sys.path.insert(0, _p)

import ml_dtypes

import concourse.bass as bass
import concourse.bacc as bacc
import concourse.mybir as mybir
import concourse.tile as tile
from concourse.bass_utils import run_bass_kernel_spmd

dt = mybir.dt
F32 = dt.float32
F32R = dt.float32r
BF16 = dt.bfloat16
AF = mybir.ActivationFunctionType
ALU = mybir.AluOpType
BF = ml_dtypes.bfloat16

L, B, E, H, D = 2048, 2, 256, 8, 32
SCALE = float(D) ** -0.5
NCORES = 8
TP = 4          # head-parallel group size (2 heads per core)
DP = 2          # batch-parallel
HPC = H // TP   # heads per core = 2
HD = HPC * D    # local head dims = 64

TQ = 512        # tq outer chunk
NJ = L // TQ    # 4
NTK = L // 128  # 16 tk chunks of 128
VW = 2 * (D + 1)  # v_buf cols per tk chunk: [v_h0 | 1 | v_h1 | 1]

_GRAPH = None


def _build_graph():
    nc = bacc.Bacc(
        "TRN2",
        target_bir_lowering=False,
        debug=False,
        enable_asserts=False,
        num_devices=NCORES,
    )

    # ---- parameters (per-core shards supplied via in_maps) ----
    xqt = nc.declare_dram_parameter("xqt", [E, L], BF16, isOutput=False).ap()
    xkt = nc.declare_dram_parameter("xkt", [E, L], BF16, isOutput=False).ap()
    xvt = nc.declare_dram_parameter("xvt", [E, L], BF16, isOutput=False).ap()
    wq = nc.declare_dram_parameter("wq", [E, HD], BF16, isOutput=False).ap()
    wk = nc.declare_dram_parameter("wk", [E, HD], BF16, isOutput=False).ap()
    wv = nc.declare_dram_parameter("wv", [E, HD], BF16, isOutput=False).ap()
    wp = nc.declare_dram_parameter("wp", [HD, E], BF16, isOutput=False).ap()
    bq = nc.declare_dram_parameter("bq", [1, HD], F32, isOutput=False).ap()
    bk = nc.declare_dram_parameter("bk", [1, HD], F32, isOutput=False).ap()
    bv = nc.declare_dram_parameter("bv", [1, HD], F32, isOutput=False).ap()
    bp = nc.declare_dram_parameter("bp", [1, E], F32, isOutput=False).ap()
    out = nc.declare_dram_parameter("out", [TQ, E], F32, isOutput=True).ap()

    with tile.TileContext(nc) as tc:
        with (
            tc.tile_pool(name="persist", bufs=1) as pp,
            tc.tile_pool(name="pt", bufs=3) as ptp,
            tc.tile_pool(name="osb", bufs=2) as osbp,
            tc.tile_pool(name="rz", bufs=4) as rzp,
            tc.tile_pool(name="outsb", bufs=2) as outp,
            tc.tile_pool(name="st", bufs=2, space="PSUM") as stp,
            tc.tile_pool(name="ot", bufs=2, space="PSUM") as otp,
            tc.tile_pool(name="pj", bufs=2, space="PSUM") as pjp,
            tc.tile_pool(name="dram", bufs=1, space="DRAM") as dramp,
        ):
            # ---------- phase 0: loads ----------
            # ACT table warm-up: load the exp table while DMAs stream.
            warm = pp.tile([1, 16], F32)
            nc.vector.memset(warm[:], 0.0)
            nc.scalar.activation(warm[:], warm[:], AF.Exp)

            # x.T tiles: [128, 2048] per E-half per tensor.
            xt_sb = []
            for name, xsrc in (("q", xqt), ("k", xkt), ("v", xvt)):
                halves = []
                for e in range(2):
                    t = pp.tile([128, L], BF16, tag=f"x{name}t{e}")
                    nc.sync.dma_start(out=t[:], in_=xsrc[e * 128:(e + 1) * 128, :])
                    halves.append(t)
                xt_sb.append(halves)
            xq_sb, xk_sb, xv_sb = xt_sb

            # weights: lhsT chunks [128, HD] per E-half, bf16
            w_sb = {}
            for name, wsrc in (("q", wq), ("k", wk), ("v", wv)):
                t = pp.tile([128, 2 * HD], BF16, tag=f"w{name}")
                for e in range(2):
                    nc.sync.dma_start(
                        out=t[:, e * HD:(e + 1) * HD],
                        in_=wsrc[e * 128:(e + 1) * 128, :],
                    )
                w_sb[name] = t
            wp_sb = pp.tile([HD, E], BF16)
            nc.sync.dma_start(out=wp_sb[:], in_=wp[:, :])

            # biases: bq/bk as per-partition columns [HD, 1]; bv/bp as rows
            bq_sb = pp.tile([HD, 1], F32)
            nc.gpsimd.dma_start(out=bq_sb[:], in_=bq.rearrange("a b -> b a"))
            bk_sb = pp.tile([HD, 1], F32)
            nc.gpsimd.dma_start(out=bk_sb[:], in_=bk.rearrange("a b -> b a"))
            bv_sb = pp.tile([128, HD], F32)
            nc.gpsimd.dma_start(out=bv_sb[:], in_=bv.to_broadcast((128, HD)))
            bp_sb = pp.tile([128, E], F32)
            nc.gpsimd.dma_start(out=bp_sb[:], in_=bp.to_broadcast((128, E)))

            # ---------- phase 1: projections ----------
            # q.T / k.T : [HD, L] f32 in SBUF (scores read them as f32r)
            qT = pp.tile([HD, L], F32R)
            kT = pp.tile([HD, L], F32R)
            for dst, wname, xsb, bias in (
                (qT, "q", xq_sb, bq_sb),
                (kT, "k", xk_sb, bk_sb),
            ):
                for n in range(L // 512):
                    ps = pjp.tile([128, 512], F32, tag="pj")
                    for e in range(2):
                        nc.tensor.matmul(
                            ps[0:HD, :],
                            w_sb[wname][:, e * HD:(e + 1) * HD],
                            xsb[e][:, n * 512:(n + 1) * 512],
                            start=(e == 0),
                            stop=(e == 1),
                        )
                    nc.vector.tensor_scalar_add(
                        dst[:, n * 512:(n + 1) * 512], ps[0:HD, :], bias[:, 0:1]
                    )

            # v_buf: [128, NTK*VW] bf16, per tk chunk [v_h0 | 1 | v_h1 | 1]
            v_buf = pp.tile([128, NTK * VW], BF16)
            nc.gpsimd.memset(v_buf[:], 1.0)
            for t in range(NTK):
                ps = pjp.tile([128, HD], F32, tag="pj")
                for e in range(2):
                    nc.tensor.matmul(
                        ps[:],
                        xv_sb[e][:, t * 128:(t + 1) * 128],
                        w_sb["v"][:, e * HD:(e + 1) * HD],
                        start=(e == 0),
                        stop=(e == 1),
                    )
                for u in range(HPC):
                    nc.vector.tensor_tensor(
                        v_buf[:, t * VW + u * (D + 1): t * VW + u * (D + 1) + D],
                        ps[:, u * D:(u + 1) * D],
                        bv_sb[:, u * D:(u + 1) * D],
                        ALU.add,
                    )

            # DRAM bounce buffers for the reduce-scatter
            bounce = dramp.tile([L, E], F32)
            rs_out = dramp.tile([TQ, E], F32)

            # ---------- phase 2: attention ----------
            for j in range(NJ):
                # unit u occupies partitions [64u, 64u+33): 32 head dims + Z
                ot = otp.tile([128, TQ], F32, tag="ot")
                for u in range(HPC):
                    for g in range(NTK // 2):
                        st = stp.tile([128, 1024], F32, tag="st")
                        for i in range(2):
                            tk = 2 * g + i
                            nc.tensor.matmul(
                                st[:, i * 512:(i + 1) * 512],
                                kT[u * D:(u + 1) * D, tk * 128:(tk + 1) * 128],
                                qT[u * D:(u + 1) * D, j * TQ:(j + 1) * TQ],
                                start=True,
                                stop=True,
                            )
                        pt = ptp.tile([128, 1024], BF16, tag="pt")
                        nc.scalar.activation(pt[:], st[:], AF.Exp, scale=SCALE)
                        for i in range(2):
                            tk = 2 * g + i
                            first = g == 0 and i == 0
                            last = g == NTK // 2 - 1 and i == 1
                            nc.tensor.matmul(
                                ot[u * 64: u * 64 + D + 1, :],
                                v_buf[:, tk * VW + u * (D + 1): tk * VW + (u + 1) * (D + 1)],
                                pt[:, i * 512:(i + 1) * 512],
                                start=first,
                                stop=last,
                                skip_group_check=True,
                            )

                # normalize: O~ = O / Z  (Z = ones-column row of each unit)
                o_sb = osbp.tile([HD, TQ], BF16, tag="osb")
                for u in range(HPC):
                    rz = rzp.tile([D, TQ], F32, tag="rz")
                    nc.vector.reciprocal(rz[0:1, :], ot[u * 64 + D: u * 64 + D + 1, :])
                    nc.gpsimd.partition_broadcast(rz[0:D, :], rz[0:1, :])
                    nc.vector.tensor_tensor(
                        o_sb[u * D:(u + 1) * D, :],
                        ot[u * 64: u * 64 + D, :],
                        rz[0:D, :],
                        ALU.mult,
                    )

                # out-proj (row-parallel partial) + bias, DMA to bounce
                for m in range(TQ // 128):
                    pj = pjp.tile([128, E], F32, tag="pj")
                    nc.tensor.matmul(
                        pj[:],
                        o_sb[:, m * 128:(m + 1) * 128],
                        wp_sb[:],
                        start=True,
                        stop=True,
                    )
                    ob = outp.tile([128, E], F32, tag="outsb")
                    nc.vector.tensor_tensor(
                        ob[:], pj[:], bp_sb[:, :], ALU.add
                    )
                    nc.sync.dma_start(
                        out=bounce[j * TQ + m * 128: j * TQ + (m + 1) * 128, :],
                        in_=ob[:],
                    )

                # chunked reduce-scatter over the TP group
                nc.gpsimd.collective_compute(
                    "ReduceScatter",
                    ALU.add,
                    replica_groups=[[0, 1, 2, 3], [4, 5, 6, 7]],
                    ins=[bounce[j * TQ:(j + 1) * TQ, :].opt()],
                    outs=[rs_out[j * 128:(j + 1) * 128, :].opt()],
                )
                nc.sync.dma_start(
                    out=out[j * 128:(j + 1) * 128, :],
                    in_=rs_out[j * 128:(j + 1) * 128, :],
                )

    return nc


def get_graph():
    global _GRAPH
    if _GRAPH is None:
        nc = _build_graph()
        nc.compile()
        _GRAPH = nc
    return _GRAPH


def make_in_maps(query, key_, value, Wq, bq, Wk, bk, Wv, bv, Wp, bp):
    query = np.asarray(query, np.float32)
    key_ = np.asarray(key_, np.float32)
    value = np.asarray(value, np.float32)
    Wq, Wk, Wv, Wp = (np.asarray(w, np.float32) for w in (Wq, Wk, Wv, Wp))
    bq, bk, bv, bp = (np.asarray(b_, np.float32) for b_ in (bq, bk, bv, bp))

    in_maps = []
    for c in range(NCORES):
        b = c // TP
        p = c % TP
        hs = slice(p * HD, (p + 1) * HD)
        m = {
            "xqt": np.ascontiguousarray(query[:, b, :].T).astype(BF),
            "xkt": np.ascontiguousarray(key_[:, b, :].T).astype(BF),
            "xvt": np.ascontiguousarray(value[:, b, :].T).astype(BF),
            "wq": np.ascontiguousarray(Wq[:, hs]).astype(BF),
            "wk": np.ascontiguousarray(Wk[:, hs]).astype(BF),
            "wv": np.ascontiguousarray(Wv[:, hs]).astype(BF),
            "wp": np.ascontiguousarray(Wp[hs, :]).astype(BF),
            "bq": bq[hs].reshape(1, HD).copy(),
            "bk": bk[hs].reshape(1, HD).copy(),
            "bv": bv[hs].reshape(1, HD).copy(),
            "bp": (bp if p == 0 else np.zeros_like(bp)).reshape(1, E).copy(),
        }
        in_maps.append(m)
    return in_maps


def assemble(results):
    out_full = np.empty((L, B, E), np.float32)
    for c in range(NCORES):
        b = c // TP
        p = c % TP
        shard = results[c]["out"]  # [TQ, E]: row 128*j+i -> tq 512*j + 128*p + i
        for j in range(NJ):
            out_full[j * TQ + p * 128: j * TQ + (p + 1) * 128, b, :] = shard[
                j * 128:(j + 1) * 128, :
            ]
    return out_full


def run(inputs, trace=False, **kw):
    nc = get_graph()
    in_maps = make_in_maps(**inputs)
    res = run_bass_kernel_spmd(
        nc, in_maps, core_ids=list(range(NCORES)), trace=trace, **kw
    )
    return res


def kernel(**inputs):
    res = run(inputs, trace=False)
    return assemble(res.results)


# revision 9
# speedup vs baseline: 1.1014x; 1.1014x over previous
"""Distributed attention kernel for 8 TRN2 NeuronCores.

Problem: L=2048, B=2, E=256, H=8 heads, D=32 head-dim, fp32.
Sharding: DP2 over batch x TP4 over heads (2 heads/core). Row-parallel
output projection with a chunked ReduceScatter over each 4-core group.

Per-core pipeline:
  phase 0: DMA in pre-transposed bf16 x.T shards + bf16 weight slices.
  phase 1: q.T/k.T = W^T @ x.T (bf16 matmul, f32 psum, bias fused in the
           psum->sbuf copy); v = value @ Wv (natural layout) interleaved
           with a ones-column for the softmax row-sum.
  phase 2: per tq-chunk(512) / head / tk-pair: S.T = k.T^T q.T in
           float32r (full-rate fp32), exp on ScalarE with fused 1/sqrt(D)
           scale (psum->sbuf, bf16), P.T @ [v|1] accumulates O.T plus the
           row-sums Z in one psum tile; normalize by 1/Z fused into the
           psum->sbuf copy; out-proj (row-parallel) + bias; DMA to a DRAM
           bounce buffer.
  phase 3: ReduceScatter(add) over the TP group per tq-chunk, DMA to out.
"""

import os
import sys

import numpy as np

for _p in ("/opt/trn_rl_repo",):
    if _p not in sys.path and os.path.isdir(_p):
        sys.path.insert(0, _p)

import ml_dtypes

import concourse.bass as bass
import concourse.bacc as bacc
import concourse.mybir as mybir
import concourse.tile as tile
from concourse.bass_utils import run_bass_kernel_spmd

dt = mybir.dt
F32 = dt.float32
F32R = dt.float32r
BF16 = dt.bfloat16
AF = mybir.ActivationFunctionType
ALU = mybir.AluOpType
BF = ml_dtypes.bfloat16

L, B, E, H, D = 2048, 2, 256, 8, 32
SCALE = float(D) ** -0.5
NCORES = 8
TP = 4          # head-parallel group size (2 heads per core)
DP = 2          # batch-parallel
HPC = H // TP   # heads per core = 2
HD = HPC * D    # local head dims = 64

TQ = 512        # tq outer chunk
NJ = L // TQ    # 4
NTK = L // 128  # 16 tk chunks of 128
VW = 2 * (D + 1)  # v_buf cols per tk chunk: [v_h0 | 1 | v_h1 | 1]

_GRAPH = None


def _build_graph():
    nc = bacc.Bacc(
        "TRN2",
        target_bir_lowering=False,
        debug=False,
        enable_asserts=False,
        num_devices=NCORES,
    )

    # ---- parameters (per-core shards supplied via in_maps) ----
    xqt = nc.declare_dram_parameter("xqt", [E, L], BF16, isOutput=False).ap()
    xkt = nc.declare_dram_parameter("xkt", [E, L], BF16, isOutput=False).ap()
    xvt = nc.declare_dram_parameter("xvt", [E, L], BF16, isOutput=False).ap()
    wq = nc.declare_dram_parameter("wq", [E, HD], BF16, isOutput=False).ap()
    wk = nc.declare_dram_parameter("wk", [E, HD], BF16, isOutput=False).ap()
    wv = nc.declare_dram_parameter("wv", [E, HD], BF16, isOutput=False).ap()
    wp = nc.declare_dram_parameter("wp", [HD, E], BF16, isOutput=False).ap()
    bq = nc.declare_dram_parameter("bq", [1, HD], F32, isOutput=False).ap()
    bk = nc.declare_dram_parameter("bk", [1, HD], F32, isOutput=False).ap()
    bv = nc.declare_dram_parameter("bv", [1, HD], F32, isOutput=False).ap()
    bp = nc.declare_dram_parameter("bp", [1, E], F32, isOutput=False).ap()
    out = nc.declare_dram_parameter("out", [TQ, E], F32, isOutput=True).ap()

    with tile.TileContext(nc) as tc:
        with (
            tc.tile_pool(name="persist", bufs=1) as pp,
            tc.tile_pool(name="pt", bufs=3) as ptp,
            tc.tile_pool(name="osb", bufs=2) as osbp,
            tc.tile_pool(name="rz", bufs=4) as rzp,
            tc.tile_pool(name="outsb", bufs=2) as outp,
            tc.tile_pool(name="st", bufs=2, space="PSUM") as stp,
            tc.tile_pool(name="ot", bufs=2, space="PSUM") as otp,
            tc.tile_pool(name="pj", bufs=2, space="PSUM") as pjp,
            tc.tile_pool(name="dram", bufs=1, space="DRAM") as dramp,
        ):
            # ---------- phase 0: loads ----------
            # ACT table warm-up: load the exp table while DMAs stream.
            warm = pp.tile([1, 16], F32)
            nc.vector.memset(warm[:], 0.0)
            nc.scalar.activation(warm[:], warm[:], AF.Exp)

            # x.T tiles: [128, 2048] per E-half per tensor.
            xt_sb = []
            for name, xsrc in (("q", xqt), ("k", xkt), ("v", xvt)):
                halves = []
                for e in range(2):
                    t = pp.tile([128, L], BF16, tag=f"x{name}t{e}")
                    nc.sync.dma_start(out=t[:], in_=xsrc[e * 128:(e + 1) * 128, :])
                    halves.append(t)
                xt_sb.append(halves)
            xq_sb, xk_sb, xv_sb = xt_sb

            # weights: lhsT chunks [128, HD] per E-half, bf16
            w_sb = {}
            for name, wsrc in (("q", wq), ("k", wk), ("v", wv)):
                t = pp.tile([128, 2 * HD], BF16, tag=f"w{name}")
                for e in range(2):
                    nc.sync.dma_start(
                        out=t[:, e * HD:(e + 1) * HD],
                        in_=wsrc[e * 128:(e + 1) * 128, :],
                    )
                w_sb[name] = t
            wp_sb = pp.tile([HD, E], BF16)
            nc.sync.dma_start(out=wp_sb[:], in_=wp[:, :])

            # biases: bq/bk as per-partition columns [HD, 1]; bv/bp as rows
            bq_sb = pp.tile([HD, 1], F32)
            nc.gpsimd.dma_start(out=bq_sb[:], in_=bq.rearrange("a b -> b a"))
            bk_sb = pp.tile([HD, 1], F32)
            nc.gpsimd.dma_start(out=bk_sb[:], in_=bk.rearrange("a b -> b a"))
            bv_sb = pp.tile([128, HD], F32)
            nc.gpsimd.dma_start(out=bv_sb[:], in_=bv.to_broadcast((128, HD)))
            bp_sb = pp.tile([128, E], F32)
            nc.gpsimd.dma_start(out=bp_sb[:], in_=bp.to_broadcast((128, E)))

            # ---------- phase 1: projections ----------
            # q.T / k.T : [HD, L] f32 in SBUF (scores read them as f32r)
            qT = pp.tile([HD, L], BF16)
            kT = pp.tile([HD, L], BF16)
            for dst, wname, xsb, bias in (
                (qT, "q", xq_sb, bq_sb),
                (kT, "k", xk_sb, bk_sb),
            ):
                for n in range(L // 512):
                    ps = pjp.tile([128, 512], F32, tag="pj")
                    for e in range(2):
                        nc.tensor.matmul(
                            ps[0:HD, :],
                            w_sb[wname][:, e * HD:(e + 1) * HD],
                            xsb[e][:, n * 512:(n + 1) * 512],
                            start=(e == 0),
                            stop=(e == 1),
                        )
                    nc.vector.tensor_scalar_add(
                        dst[:, n * 512:(n + 1) * 512], ps[0:HD, :], bias[:, 0:1]
                    )

            # v_buf: [128, NTK*VW] bf16, per tk chunk [v_h0 | 1 | v_h1 | 1]
            v_buf = pp.tile([128, NTK * VW], BF16)
            nc.gpsimd.memset(v_buf[:], 1.0)
            for t in range(NTK):
                ps = pjp.tile([128, HD], F32, tag="pj")
                for e in range(2):
                    nc.tensor.matmul(
                        ps[:],
                        xv_sb[e][:, t * 128:(t + 1) * 128],
                        w_sb["v"][:, e * HD:(e + 1) * HD],
                        start=(e == 0),
                        stop=(e == 1),
                    )
                for u in range(HPC):
                    nc.vector.tensor_tensor(
                        v_buf[:, t * VW + u * (D + 1): t * VW + u * (D + 1) + D],
                        ps[:, u * D:(u + 1) * D],
                        bv_sb[:, u * D:(u + 1) * D],
                        ALU.add,
                    )

            # DRAM bounce buffers for the reduce-scatter
            bounce = dramp.tile([L, E], F32)
            rs_out = dramp.tile([TQ, E], F32)

            # ---------- phase 2: attention ----------
            for j in range(NJ):
                # unit u occupies partitions [64u, 64u+33): 32 head dims + Z
                ot = otp.tile([128, TQ], F32, tag="ot")
                for u in range(HPC):
                    for g in range(NTK // 2):
                        st = stp.tile([128, 1024], F32, tag="st")
                        for i in range(2):
                            tk = 2 * g + i
                            nc.tensor.matmul(
                                st[:, i * 512:(i + 1) * 512],
                                kT[u * D:(u + 1) * D, tk * 128:(tk + 1) * 128],
                                qT[u * D:(u + 1) * D, j * TQ:(j + 1) * TQ],
                                start=True,
                                stop=True,
                            )
                        pt = ptp.tile([128, 1024], BF16, tag="pt")
                        nc.scalar.activation(pt[:], st[:], AF.Exp, scale=SCALE)
                        for i in range(2):
                            tk = 2 * g + i
                            first = g == 0 and i == 0
                            last = g == NTK // 2 - 1 and i == 1
                            nc.tensor.matmul(
                                ot[u * 64: u * 64 + D + 1, :],
                                v_buf[:, tk * VW + u * (D + 1): tk * VW + (u + 1) * (D + 1)],
                                pt[:, i * 512:(i + 1) * 512],
                                start=first,
                                stop=last,
                                skip_group_check=True,
                            )

                # normalize: O~ = O / Z  (Z = ones-column row of each unit)
                o_sb = osbp.tile([HD, TQ], BF16, tag="osb")
                for u in range(HPC):
                    rz = rzp.tile([D, TQ], F32, tag="rz")
                    nc.vector.reciprocal(rz[0:1, :], ot[u * 64 + D: u * 64 + D + 1, :])
                    nc.gpsimd.partition_broadcast(rz[0:D, :], rz[0:1, :])
                    nc.vector.tensor_tensor(
                        o_sb[u * D:(u + 1) * D, :],
                        ot[u * 64: u * 64 + D, :],
                        rz[0:D, :],
                        ALU.mult,
                    )

                # out-proj (row-parallel partial) + bias, DMA to bounce
                for m in range(TQ // 128):
                    pj = pjp.tile([128, E], F32, tag="pj")
                    nc.tensor.matmul(
                        pj[:],
                        o_sb[:, m * 128:(m + 1) * 128],
                        wp_sb[:],
                        start=True,
                        stop=True,
                    )
                    ob = outp.tile([128, E], F32, tag="outsb")
                    nc.vector.tensor_tensor(
                        ob[:], pj[:], bp_sb[:, :], ALU.add
                    )
                    nc.sync.dma_start(
                        out=bounce[j * TQ + m * 128: j * TQ + (m + 1) * 128, :],
                        in_=ob[:],
                    )

                # chunked reduce-scatter over the TP group
                nc.gpsimd.collective_compute(
                    "ReduceScatter",
                    ALU.add,
                    replica_groups=[[0, 1, 2, 3], [4, 5, 6, 7]],
                    ins=[bounce[j * TQ:(j + 1) * TQ, :].opt()],
                    outs=[rs_out[j * 128:(j + 1) * 128, :].opt()],
                )
                nc.sync.dma_start(
                    out=out[j * 128:(j + 1) * 128, :],
                    in_=rs_out[j * 128:(j + 1) * 128, :],
                )

    return nc


def get_graph():
    global _GRAPH
    if _GRAPH is None:
        nc = _build_graph()
        nc.compile()
        _GRAPH = nc
    return _GRAPH


def make_in_maps(query, key_, value, Wq, bq, Wk, bk, Wv, bv, Wp, bp):
    query = np.asarray(query, np.float32)
    key_ = np.asarray(key_, np.float32)
    value = np.asarray(value, np.float32)
    Wq, Wk, Wv, Wp = (np.asarray(w, np.float32) for w in (Wq, Wk, Wv, Wp))
    bq, bk, bv, bp = (np.asarray(b_, np.float32) for b_ in (bq, bk, bv, bp))

    in_maps = []
    for c in range(NCORES):
        b = c // TP
        p = c % TP
        hs = slice(p * HD, (p + 1) * HD)
        m = {
            "xqt": np.ascontiguousarray(query[:, b, :].T).astype(BF),
            "xkt": np.ascontiguousarray(key_[:, b, :].T).astype(BF),
            "xvt": np.ascontiguousarray(value[:, b, :].T).astype(BF),
            "wq": np.ascontiguousarray(Wq[:, hs]).astype(BF),
            "wk": np.ascontiguousarray(Wk[:, hs]).astype(BF),
            "wv": np.ascontiguousarray(Wv[:, hs]).astype(BF),
            "wp": np.ascontiguousarray(Wp[hs, :]).astype(BF),
            "bq": bq[hs].reshape(1, HD).copy(),
            "bk": bk[hs].reshape(1, HD).copy(),
            "bv": bv[hs].reshape(1, HD).copy(),
            "bp": (bp if p == 0 else np.zeros_like(bp)).reshape(1, E).copy(),
        }
        in_maps.append(m)
    return in_maps


def assemble(results):
    out_full = np.empty((L, B, E), np.float32)
    for c in range(NCORES):
        b = c // TP
        p = c % TP
        shard = results[c]["out"]  # [TQ, E]: row 128*j+i -> tq 512*j + 128*p + i
        for j in range(NJ):
            out_full[j * TQ + p * 128: j * TQ + (p + 1) * 128, b, :] = shard[
                j * 128:(j + 1) * 128, :
            ]
    return out_full


def run(inputs, trace=False, **kw):
    nc = get_graph()
    in_maps = make_in_maps(**inputs)
    res = run_bass_kernel_spmd(
        nc, in_maps, core_ids=list(range(NCORES)), trace=trace, **kw
    )
    return res


def kernel(**inputs):
    res = run(inputs, trace=False)
    return assemble(res.results)


# revision 10
# speedup vs baseline: 1.1758x; 1.0675x over previous
"""Distributed attention kernel for 8 TRN2 NeuronCores.

Problem: L=2048, B=2, E=256, H=8 heads, D=32 head-dim, fp32.
Sharding: DP2 over batch x TP4 over heads (2 heads/core). Row-parallel
output projection with a chunked ReduceScatter over each 4-core group.

Per-core pipeline:
  phase 0: DMA in pre-transposed bf16 x.T shards + bf16 weight slices.
  phase 1: q.T/k.T = W^T @ x.T (bf16 matmul, f32 psum, bias fused in the
           psum->sbuf copy); v = value @ Wv (natural layout) interleaved
           with a ones-column for the softmax row-sum.
  phase 2: per tq-chunk(512) / head / tk-pair: S.T = k.T^T q.T in
           float32r (full-rate fp32), exp on ScalarE with fused 1/sqrt(D)
           scale (psum->sbuf, bf16), P.T @ [v|1] accumulates O.T plus the
           row-sums Z in one psum tile; normalize by 1/Z fused into the
           psum->sbuf copy; out-proj (row-parallel) + bias; DMA to a DRAM
           bounce buffer.
  phase 3: ReduceScatter(add) over the TP group per tq-chunk, DMA to out.
"""

import os
import sys

import numpy as np

for _p in ("/opt/trn_rl_repo",):
    if _p not in sys.path and os.path.isdir(_p):
        sys.path.insert(0, _p)

import ml_dtypes

import concourse.bass as bass
import concourse.bacc as bacc
import concourse.mybir as mybir
import concourse.tile as tile
from concourse.bass_utils import run_bass_kernel_spmd

dt = mybir.dt
F32 = dt.float32
F32R = dt.float32r
BF16 = dt.bfloat16
AF = mybir.ActivationFunctionType
ALU = mybir.AluOpType
BF = ml_dtypes.bfloat16

L, B, E, H, D = 2048, 2, 256, 8, 32
SCALE = float(D) ** -0.5
NCORES = 8
TP = 4          # head-parallel group size (2 heads per core)
DP = 2          # batch-parallel
HPC = H // TP   # heads per core = 2
HD = HPC * D    # local head dims = 64

TQ = 512        # tq outer chunk
NJ = L // TQ    # 4
NTK = L // 128  # 16 tk chunks of 128
VW = 2 * (D + 1)  # v_buf cols per tk chunk: [v_h0 | 1 | v_h1 | 1]

_GRAPH = None


def _build_graph():
    nc = bacc.Bacc(
        "TRN2",
        target_bir_lowering=False,
        debug=False,
        enable_asserts=False,
        num_devices=NCORES,
    )

    # ---- parameters (per-core shards supplied via in_maps) ----
    xqt = nc.declare_dram_parameter("xqt", [E, L], BF16, isOutput=False).ap()
    xkt = nc.declare_dram_parameter("xkt", [E, L], BF16, isOutput=False).ap()
    xvt = nc.declare_dram_parameter("xvt", [E, L], BF16, isOutput=False).ap()
    wq = nc.declare_dram_parameter("wq", [E, HD], BF16, isOutput=False).ap()
    wk = nc.declare_dram_parameter("wk", [E, HD], BF16, isOutput=False).ap()
    wv = nc.declare_dram_parameter("wv", [E, HD], BF16, isOutput=False).ap()
    wp = nc.declare_dram_parameter("wp", [HD, E], BF16, isOutput=False).ap()
    bq = nc.declare_dram_parameter("bq", [1, HD], F32, isOutput=False).ap()
    bk = nc.declare_dram_parameter("bk", [1, HD], F32, isOutput=False).ap()
    bv = nc.declare_dram_parameter("bv", [1, HD], F32, isOutput=False).ap()
    bp = nc.declare_dram_parameter("bp", [1, E], F32, isOutput=False).ap()
    out = nc.declare_dram_parameter("out", [TQ, E], F32, isOutput=True).ap()

    with tile.TileContext(nc) as tc:
        with (
            tc.tile_pool(name="persist", bufs=1) as pp,
            tc.tile_pool(name="pt", bufs=3) as ptp,
            tc.tile_pool(name="osb", bufs=2) as osbp,
            tc.tile_pool(name="rz", bufs=4) as rzp,
            tc.tile_pool(name="outsb", bufs=2) as outp,
            tc.tile_pool(name="st", bufs=2, space="PSUM") as stp,
            tc.tile_pool(name="ot", bufs=2, space="PSUM") as otp,
            tc.tile_pool(name="pj", bufs=2, space="PSUM") as pjp,
            tc.tile_pool(name="dram", bufs=1, space="DRAM") as dramp,
        ):
            # ---------- phase 0: loads ----------
            # ACT table warm-up: load the exp table while DMAs stream.
            warm = pp.tile([1, 16], F32)
            nc.vector.memset(warm[:], 0.0)
            nc.scalar.activation(warm[:], warm[:], AF.Exp)

            # x.T tiles: [128, 2048] per E-half per tensor.
            xt_sb = []
            for name, xsrc in (("q", xqt), ("k", xkt), ("v", xvt)):
                halves = []
                for e in range(2):
                    t = pp.tile([128, L], BF16, tag=f"x{name}t{e}")
                    for n in range(L // 512):
                        nc.sync.dma_start(
                            out=t[:, n * 512:(n + 1) * 512],
                            in_=xsrc[e * 128:(e + 1) * 128, n * 512:(n + 1) * 512],
                        )
                    halves.append(t)
                xt_sb.append(halves)
            xq_sb, xk_sb, xv_sb = xt_sb

            # weights: lhsT chunks [128, HD] per E-half, bf16
            w_sb = {}
            for name, wsrc in (("q", wq), ("k", wk), ("v", wv)):
                t = pp.tile([128, 2 * HD], BF16, tag=f"w{name}")
                for e in range(2):
                    nc.sync.dma_start(
                        out=t[:, e * HD:(e + 1) * HD],
                        in_=wsrc[e * 128:(e + 1) * 128, :],
                    )
                w_sb[name] = t
            wp_sb = pp.tile([HD, E], BF16)
            nc.sync.dma_start(out=wp_sb[:], in_=wp[:, :])

            # biases: bq/bk as per-partition columns [HD, 1]; bv/bp as rows
            bq_sb = pp.tile([HD, 1], F32)
            nc.gpsimd.dma_start(out=bq_sb[:], in_=bq.rearrange("a b -> b a"))
            bk_sb = pp.tile([HD, 1], F32)
            nc.gpsimd.dma_start(out=bk_sb[:], in_=bk.rearrange("a b -> b a"))
            bv_sb = pp.tile([128, HD], F32)
            nc.gpsimd.dma_start(out=bv_sb[:], in_=bv.to_broadcast((128, HD)))
            bp_sb = pp.tile([128, E], F32)
            nc.gpsimd.dma_start(out=bp_sb[:], in_=bp.to_broadcast((128, E)))

            # ---------- phase 1: projections ----------
            # q.T / k.T : [HD, L] f32 in SBUF (scores read them as f32r)
            qT = pp.tile([HD, L], BF16)
            kT = pp.tile([HD, L], BF16)
            for dst, wname, xsb, bias in (
                (kT, "k", xk_sb, bk_sb),
                (qT, "q", xq_sb, bq_sb),
            ):
                for n in range(L // 512):
                    ps = pjp.tile([128, 512], F32, tag="pj")
                    for e in range(2):
                        nc.tensor.matmul(
                            ps[0:HD, :],
                            w_sb[wname][:, e * HD:(e + 1) * HD],
                            xsb[e][:, n * 512:(n + 1) * 512],
                            start=(e == 0),
                            stop=(e == 1),
                        )
                    nc.vector.tensor_scalar_add(
                        dst[:, n * 512:(n + 1) * 512], ps[0:HD, :], bias[:, 0:1]
                    )

            # v_buf: [128, NTK*VW] bf16, per tk chunk [v_h0 | 1 | v_h1 | 1]
            v_buf = pp.tile([128, NTK * VW], BF16)
            nc.gpsimd.memset(v_buf[:], 1.0)
            for t in range(NTK):
                ps = pjp.tile([128, HD], F32, tag="pj")
                for e in range(2):
                    nc.tensor.matmul(
                        ps[:],
                        xv_sb[e][:, t * 128:(t + 1) * 128],
                        w_sb["v"][:, e * HD:(e + 1) * HD],
                        start=(e == 0),
                        stop=(e == 1),
                    )
                for u in range(HPC):
                    nc.vector.tensor_tensor(
                        v_buf[:, t * VW + u * (D + 1): t * VW + u * (D + 1) + D],
                        ps[:, u * D:(u + 1) * D],
                        bv_sb[:, u * D:(u + 1) * D],
                        ALU.add,
                    )

            # DRAM bounce buffers for the reduce-scatter (bf16 to halve bytes)
            bounce = dramp.tile([L, E], BF16)
            rs_out = dramp.tile([TQ, E], BF16)

            # ---------- phase 2: attention ----------
            for j in range(NJ):
                # unit u occupies partitions [64u, 64u+33): 32 head dims + Z
                ot = otp.tile([128, TQ], F32, tag="ot")
                o_sb = osbp.tile([HD, TQ], BF16, tag="osb")
                for u in range(HPC):
                    for g in range(NTK // 2):
                        st = stp.tile([128, 1024], F32, tag="st")
                        for i in range(2):
                            tk = 2 * g + i
                            nc.tensor.matmul(
                                st[:, i * 512:(i + 1) * 512],
                                kT[u * D:(u + 1) * D, tk * 128:(tk + 1) * 128],
                                qT[u * D:(u + 1) * D, j * TQ:(j + 1) * TQ],
                                start=True,
                                stop=True,
                            )
                        pt = ptp.tile([128, 1024], BF16, tag="pt")
                        nc.scalar.activation(pt[:], st[:], AF.Exp, scale=SCALE)
                        for i in range(2):
                            tk = 2 * g + i
                            first = g == 0 and i == 0
                            last = g == NTK // 2 - 1 and i == 1
                            nc.tensor.matmul(
                                ot[u * 64: u * 64 + D + 1, :],
                                v_buf[:, tk * VW + u * (D + 1): tk * VW + (u + 1) * (D + 1)],
                                pt[:, i * 512:(i + 1) * 512],
                                start=first,
                                stop=last,
                                skip_group_check=True,
                            )

                    # normalize unit u as soon as its PV sweep finishes:
                    # O~ = O / Z (Z = ones-column row)
                    rz = rzp.tile([D, TQ], F32, tag="rz")
                    nc.vector.reciprocal(rz[0:1, :], ot[u * 64 + D: u * 64 + D + 1, :])
                    nc.gpsimd.partition_broadcast(rz[0:D, :], rz[0:1, :])
                    nc.vector.tensor_tensor(
                        o_sb[u * D:(u + 1) * D, :],
                        ot[u * 64: u * 64 + D, :],
                        rz[0:D, :],
                        ALU.mult,
                    )

                # out-proj (row-parallel partial) + bias, DMA to bounce
                for m in range(TQ // 128):
                    pj = pjp.tile([128, E], F32, tag="pj")
                    nc.tensor.matmul(
                        pj[:],
                        o_sb[:, m * 128:(m + 1) * 128],
                        wp_sb[:],
                        start=True,
                        stop=True,
                    )
                    ob = outp.tile([128, E], BF16, tag="outsb")
                    nc.vector.tensor_tensor(
                        ob[:], pj[:], bp_sb[:, :], ALU.add
                    )
                    nc.sync.dma_start(
                        out=bounce[j * TQ + m * 128: j * TQ + (m + 1) * 128, :],
                        in_=ob[:],
                    )

                # chunked reduce-scatter over the TP group
                nc.gpsimd.collective_compute(
                    "ReduceScatter",
                    ALU.add,
                    replica_groups=[[0, 1, 2, 3], [4, 5, 6, 7]],
                    ins=[bounce[j * TQ:(j + 1) * TQ, :].opt()],
                    outs=[rs_out[j * 128:(j + 1) * 128, :].opt()],
                )
                nc.gpsimd.dma_start(
                    out=out[j * 128:(j + 1) * 128, :],
                    in_=rs_out[j * 128:(j + 1) * 128, :],
                )

    return nc


def get_graph():
    global _GRAPH
    if _GRAPH is None:
        nc = _build_graph()
        nc.compile()
        _GRAPH = nc
    return _GRAPH


def make_in_maps(query, key_, value, Wq, bq, Wk, bk, Wv, bv, Wp, bp):
    query = np.asarray(query, np.float32)
    key_ = np.asarray(key_, np.float32)
    value = np.asarray(value, np.float32)
    Wq, Wk, Wv, Wp = (np.asarray(w, np.float32) for w in (Wq, Wk, Wv, Wp))
    bq, bk, bv, bp = (np.asarray(b_, np.float32) for b_ in (bq, bk, bv, bp))

    in_maps = []
    for c in range(NCORES):
        b = c // TP
        p = c % TP
        hs = slice(p * HD, (p + 1) * HD)
        m = {
            "xqt": np.ascontiguousarray(query[:, b, :].T).astype(BF),
            "xkt": np.ascontiguousarray(key_[:, b, :].T).astype(BF),
            "xvt": np.ascontiguousarray(value[:, b, :].T).astype(BF),
            "wq": np.ascontiguousarray(Wq[:, hs]).astype(BF),
            "wk": np.ascontiguousarray(Wk[:, hs]).astype(BF),
            "wv": np.ascontiguousarray(Wv[:, hs]).astype(BF),
            "wp": np.ascontiguousarray(Wp[hs, :]).astype(BF),
            "bq": bq[hs].reshape(1, HD).copy(),
            "bk": bk[hs].reshape(1, HD).copy(),
            "bv": bv[hs].reshape(1, HD).copy(),
            "bp": (bp if p == 0 else np.zeros_like(bp)).reshape(1, E).copy(),
        }
        in_maps.append(m)
    return in_maps


def assemble(results):
    out_full = np.empty((L, B, E), np.float32)
    for c in range(NCORES):
        b = c // TP
        p = c % TP
        shard = results[c]["out"]  # [TQ, E]: row 128*j+i -> tq 512*j + 128*p + i
        for j in range(NJ):
            out_full[j * TQ + p * 128: j * TQ + (p + 1) * 128, b, :] = shard[
                j * 128:(j + 1) * 128, :
            ]
    return out_full


def run(inputs, trace=False, **kw):
    nc = get_graph()
    in_maps = make_in_maps(**inputs)
    res = run_bass_kernel_spmd(
        nc, in_maps, core_ids=list(range(NCORES)), trace=trace, **kw
    )
    return res


def kernel(**inputs):
    res = run(inputs, trace=False)
    return assemble(res.results)


# revision 13
# speedup vs baseline: 1.2937x; 1.1003x over previous
"""Distributed attention kernel for 8 TRN2 NeuronCores.

Problem: L=2048, B=2, E=256, H=8 heads, D=32 head-dim, fp32.

Sharding: DP2 over batch x sequence-parallel-4 over query positions.
Core c handles batch c//4, query rows [512*(c%4), 512*(c%4+1)), ALL 8
heads. k/v projections are redundantly computed per batch group (cheap)
and NO collective is needed: each core owns a disjoint output block.

Per-core pipeline:
  phase 0: DMA in bf16 x.T shards (k/v full batch, q slice) + weights.
  phase 1: k.T (all heads) = Wk^T x.T; q.T slice; v (natural layout) via
           a bf16 staging tile + strided SBUF->SBUF DMA into per-head
           [v|1] slots (ones column gives the softmax row-sum for free).
  phase 2: two passes of 4 heads; per (head, tk-pair): S.T = k.T^T q.T
           (bf16, f32 psum), exp on ScalarE with fused 1/sqrt(D) scale
           -> P.T bf16, P.T @ [v|1] accumulates O.T + Z; per head:
           1/Z broadcast + normalize into o_sb (bf16).
  phase 3: out-proj over all 256 head-dims (2 accumulated matmuls per
           tq chunk) + bias, DMA straight to the output. No collective.
"""

import os
import sys

import numpy as np

for _p in ("/opt/trn_rl_repo",):
    if _p not in sys.path and os.path.isdir(_p):
        sys.path.insert(0, _p)

import ml_dtypes

import concourse.bass as bass
import concourse.bacc as bacc
import concourse.mybir as mybir
import concourse.tile as tile
from concourse.bass_utils import run_bass_kernel_spmd

dt = mybir.dt
F32 = dt.float32
BF16 = dt.bfloat16
AF = mybir.ActivationFunctionType
ALU = mybir.AluOpType
BF = ml_dtypes.bfloat16

L, B, E, H, D = 2048, 2, 256, 8, 32
SCALE = float(D) ** -0.5
NCORES = 8
SP = 4            # sequence-parallel ways
TQ = L // SP      # 512 query rows per core
NTK = L // 128    # 16 tk chunks
VW = H * (D + 1)  # v_buf cols per tk chunk: 8x [v_h | 1] = 264
NPASS = 2         # head passes (4 heads each)

_GRAPH = None


def _build_graph():
    nc = bacc.Bacc(
        "TRN2",
        target_bir_lowering=False,
        debug=False,
        enable_asserts=False,
        num_devices=NCORES,
    )

    xqt = nc.declare_dram_parameter("xqt", [E, TQ], BF16, isOutput=False).ap()
    xkt = nc.declare_dram_parameter("xkt", [E, L], BF16, isOutput=False).ap()
    xvt = nc.declare_dram_parameter("xvt", [E, L], BF16, isOutput=False).ap()
    wq = nc.declare_dram_parameter("wq", [E, E], BF16, isOutput=False).ap()
    wk = nc.declare_dram_parameter("wk", [E, E], BF16, isOutput=False).ap()
    wv = nc.declare_dram_parameter("wv", [E, E], BF16, isOutput=False).ap()
    wp = nc.declare_dram_parameter("wp", [E, E], BF16, isOutput=False).ap()
    bq = nc.declare_dram_parameter("bq", [1, E], F32, isOutput=False).ap()
    bk = nc.declare_dram_parameter("bk", [1, E], F32, isOutput=False).ap()
    bv = nc.declare_dram_parameter("bv", [1, E], F32, isOutput=False).ap()
    bp = nc.declare_dram_parameter("bp", [1, E], F32, isOutput=False).ap()
    out = nc.declare_dram_parameter("out", [TQ, E], F32, isOutput=True).ap()

    with tile.TileContext(nc) as tc:
        with (
            tc.tile_pool(name="persist", bufs=1) as pp,
            tc.tile_pool(name="pt", bufs=3) as ptp,
            tc.tile_pool(name="osb", bufs=2) as osbp,
            tc.tile_pool(name="rz", bufs=4) as rzp,
            tc.tile_pool(name="vstage", bufs=3) as vsp,
            tc.tile_pool(name="outsb", bufs=2) as outp,
            tc.tile_pool(name="st", bufs=2, space="PSUM") as stp,
            tc.tile_pool(name="ot", bufs=2, space="PSUM") as otp,
            tc.tile_pool(name="pj", bufs=2, space="PSUM") as pjp,
        ):
            # ---------- phase 0: loads ----------
            warm = pp.tile([1, 16], F32)
            nc.vector.memset(warm[:], 0.0)
            nc.scalar.activation(warm[:], warm[:], AF.Exp)

            # k/v x.T: [128, 2048] per E-half, chunked for early start
            xk_sb, xv_sb = [], []
            for name, xsrc, lst in (("k", xkt, xk_sb), ("v", xvt, xv_sb)):
                for e in range(2):
                    t = pp.tile([128, L], BF16, tag=f"x{name}t{e}")
                    for n in range(L // 512):
                        nc.sync.dma_start(
                            out=t[:, n * 512:(n + 1) * 512],
                            in_=xsrc[e * 128:(e + 1) * 128, n * 512:(n + 1) * 512],
                        )
                    lst.append(t)
            xq_sb = []
            for e in range(2):
                t = pp.tile([128, TQ], BF16, tag=f"xqt{e}")
                nc.sync.dma_start(out=t[:], in_=xqt[e * 128:(e + 1) * 128, :])
                xq_sb.append(t)

            # weights: tile [128, 2E]; slice e covers W rows [128e, 128e+128)
            w_sb = {}
            for name, wsrc in (("k", wk), ("q", wq), ("v", wv), ("p", wp)):
                t = pp.tile([128, 2 * E], BF16, tag=f"w{name}")
                for e in range(2):
                    nc.sync.dma_start(
                        out=t[:, e * E:(e + 1) * E],
                        in_=wsrc[e * 128:(e + 1) * 128, :],
                    )
                w_sb[name] = t

            # biases: bq/bk as per-partition columns [128, 2] (hc chunks);
            # bv/bp replicated across partitions
            bq_sb = pp.tile([128, 2], F32)
            nc.gpsimd.dma_start(
                out=bq_sb[:], in_=bq.rearrange("a (c p) -> p (a c)", p=128)
            )
            bk_sb = pp.tile([128, 2], F32)
            nc.gpsimd.dma_start(
                out=bk_sb[:], in_=bk.rearrange("a (c p) -> p (a c)", p=128)
            )
            bv_sb = pp.tile([128, E], F32)
            nc.gpsimd.dma_start(out=bv_sb[:], in_=bv.to_broadcast((128, E)))
            bp_sb = pp.tile([128, E], F32)
            nc.gpsimd.dma_start(out=bp_sb[:], in_=bp.to_broadcast((128, E)))

            # ---------- phase 1: projections ----------
            # k.T: [256 head-dims, 2048] as four [64, 2048] tiles
            # (2 heads per tile at partition bases 0/32 - PE requires
            # lhsT/rhs base partitions in {0, 32, 64})
            kT = [pp.tile([64, L], BF16, name=f"kT{pc}", tag=f"kT{pc}")
                  for pc in range(4)]
            for hc in range(2):
                for n in range(L // 512):
                    ps = pjp.tile([128, 512], F32, tag="pj")
                    for e in range(2):
                        nc.tensor.matmul(
                            ps[:],
                            w_sb["k"][:, e * E + hc * 128: e * E + (hc + 1) * 128],
                            xk_sb[e][:, n * 512:(n + 1) * 512],
                            start=(e == 0),
                            stop=(e == 1),
                        )
                    for half in range(2):
                        nc.vector.tensor_scalar_add(
                            kT[2 * hc + half][:, n * 512:(n + 1) * 512],
                            ps[half * 64:(half + 1) * 64, :],
                            bk_sb[half * 64:(half + 1) * 64, hc:hc + 1],
                        )

            # q.T slice: four [64, 512] tiles
            qT = [pp.tile([64, TQ], BF16, name=f"qT{pc}", tag=f"qT{pc}")
                  for pc in range(4)]
            for hc in range(2):
                ps = pjp.tile([128, 512], F32, tag="pj")
                for e in range(2):
                    nc.tensor.matmul(
                        ps[:],
                        w_sb["q"][:, e * E + hc * 128: e * E + (hc + 1) * 128],
                        xq_sb[e][:, :],
                        start=(e == 0),
                        stop=(e == 1),
                    )
                for half in range(2):
                    nc.vector.tensor_scalar_add(
                        qT[2 * hc + half][:, :],
                        ps[half * 64:(half + 1) * 64, :],
                        bq_sb[half * 64:(half + 1) * 64, hc:hc + 1],
                    )

            # v_buf: per tk chunk, 8x [v_h (32) | 1] slots
            v_buf = pp.tile([128, NTK * VW], BF16)
            nc.gpsimd.memset(v_buf[:], 1.0)
            for t in range(NTK):
                ps = pjp.tile([128, E], F32, tag="pj")
                for e in range(2):
                    nc.tensor.matmul(
                        ps[:],
                        xv_sb[e][:, t * 128:(t + 1) * 128],
                        w_sb["v"][:, e * E:(e + 1) * E],
                        start=(e == 0),
                        stop=(e == 1),
                    )
                vs = vsp.tile([128, E], BF16, tag="vstage")
                nc.vector.tensor_tensor(vs[:], ps[:], bv_sb[:], ALU.add)
                # scatter the 8 heads' 32-col blocks into the [v|1] slots
                nc.sync.dma_start(
                    out=v_buf[:, t * VW:(t + 1) * VW].rearrange(
                        "p (h w) -> p h w", h=H
                    )[:, :, 0:D],
                    in_=vs[:].rearrange("p (h d) -> p h d", h=H),
                )

            # ---------- phase 2: attention (2 passes of 4 heads) ----------
            o_sb = []
            for p in range(NPASS):
                osb = osbp.tile([128, TQ], BF16, tag="osb")
                o_sb.append(osb)
                for hh in range(2):  # two head-pairs per pass
                    ot = otp.tile([128, TQ], F32, tag="ot")
                    for u in range(2):  # head within pair: psum base 64*u
                        h = p * 4 + hh * 2 + u
                        hc, hr = h // 2, (h % 2) * D
                        for g in range(NTK // 2):
                            st = stp.tile([128, 1024], F32, tag="st")
                            for i in range(2):
                                tk = 2 * g + i
                                nc.tensor.matmul(
                                    st[:, i * 512:(i + 1) * 512],
                                    kT[hc][hr:hr + D, tk * 128:(tk + 1) * 128],
                                    qT[hc][hr:hr + D, :],
                                    start=True,
                                    stop=True,
                                )
                            pt = ptp.tile([128, 1024], BF16, tag="pt")
                            nc.scalar.activation(pt[:], st[:], AF.Exp, scale=SCALE)
                            for i in range(2):
                                tk = 2 * g + i
                                nc.tensor.matmul(
                                    ot[u * 64: u * 64 + D + 1, :],
                                    v_buf[:, tk * VW + h * (D + 1): tk * VW + (h + 1) * (D + 1)],
                                    pt[:, i * 512:(i + 1) * 512],
                                    start=(g == 0 and i == 0),
                                    stop=(g == NTK // 2 - 1 and i == 1),
                                    skip_group_check=True,
                                )

                        # normalize head h: O~ = O / Z (Z = ones-column row)
                        rz = rzp.tile([D, TQ], F32, tag="rz")
                        nc.vector.reciprocal(
                            rz[0:1, :], ot[u * 64 + D: u * 64 + D + 1, :]
                        )
                        nc.gpsimd.partition_broadcast(rz[0:D, :], rz[0:1, :])
                        nc.vector.tensor_tensor(
                            osb[(hh * 2 + u) * D:(hh * 2 + u + 1) * D, :],
                            ot[u * 64: u * 64 + D, :],
                            rz[0:D, :],
                            ALU.mult,
                        )

            # ---------- phase 3: out-proj + bias + DMA out ----------
            for m in range(TQ // 128):
                pj = pjp.tile([128, E], F32, tag="pj")
                for p in range(NPASS):
                    nc.tensor.matmul(
                        pj[:],
                        o_sb[p][:, m * 128:(m + 1) * 128],
                        w_sb["p"][:, p * E:(p + 1) * E],
                        start=(p == 0),
                        stop=(p == NPASS - 1),
                    )
                ob = outp.tile([128, E], F32, tag="outsb")
                nc.vector.tensor_tensor(ob[:], pj[:], bp_sb[:], ALU.add)
                nc.sync.dma_start(
                    out=out[m * 128:(m + 1) * 128, :], in_=ob[:]
                )

    return nc


def get_graph():
    global _GRAPH
    if _GRAPH is None:
        nc = _build_graph()
        nc.compile()
        _GRAPH = nc
    return _GRAPH


def make_in_maps(query, key_, value, Wq, bq, Wk, bk, Wv, bv, Wp, bp):
    query = np.asarray(query, np.float32)
    key_ = np.asarray(key_, np.float32)
    value = np.asarray(value, np.float32)
    Wq, Wk, Wv, Wp = (np.asarray(w, np.float32) for w in (Wq, Wk, Wv, Wp))
    bq, bk, bv, bp = (np.asarray(b_, np.float32) for b_ in (bq, bk, bv, bp))

    wq_b = np.ascontiguousarray(Wq).astype(BF)
    wk_b = np.ascontiguousarray(Wk).astype(BF)
    wv_b = np.ascontiguousarray(Wv).astype(BF)
    wp_b = np.ascontiguousarray(Wp).astype(BF)
    xt = {}
    for b in range(B):
        xt[("q", b)] = np.ascontiguousarray(query[:, b, :].T).astype(BF)
        xt[("k", b)] = np.ascontiguousarray(key_[:, b, :].T).astype(BF)
        xt[("v", b)] = np.ascontiguousarray(value[:, b, :].T).astype(BF)

    in_maps = []
    for c in range(NCORES):
        b = c // SP
        p = c % SP
        m = {
            "xqt": np.ascontiguousarray(xt[("q", b)][:, p * TQ:(p + 1) * TQ]),
            "xkt": xt[("k", b)],
            "xvt": xt[("v", b)],
            "wq": wq_b,
            "wk": wk_b,
            "wv": wv_b,
            "wp": wp_b,
            "bq": bq.reshape(1, E).copy(),
            "bk": bk.reshape(1, E).copy(),
            "bv": bv.reshape(1, E).copy(),
            "bp": bp.reshape(1, E).copy(),
        }
        in_maps.append(m)
    return in_maps


def assemble(results):
    out_full = np.empty((L, B, E), np.float32)
    for c in range(NCORES):
        b = c // SP
        p = c % SP
        out_full[p * TQ:(p + 1) * TQ, b, :] = results[c]["out"]
    return out_full


def run(inputs, trace=False, **kw):
    nc = get_graph()
    in_maps = make_in_maps(**inputs)
    res = run_bass_kernel_spmd(
        nc, in_maps, core_ids=list(range(NCORES)), trace=trace, **kw
    )
    return res


def kernel(**inputs):
    res = run(inputs, trace=False)
    return assemble(res.results)
